# revision 13
# baseline (speedup 1.0000x reference)
"""Single-head self-attention (B=8, S=2048, D=K=V=1024) on 8 TRN2 NeuronCores.

Sharding: data-parallel over batch — one batch element per core. Each core
computes its full attention layer; no collectives.

Per-core dataflow (all matmuls bf16 with fp32 PSUM accumulation):
  phase 1: x --PE-transpose--> xT[d,s] (bf16)
           qT[f,s] = Wq^T xT   (feature-major, + bias via ScalarE)
           kT[f,s] = Wk^T xT
           v[s,f]  = x Wv      (natural layout, + bias via VectorE)
  phase 2: per i-block of 512 queries:
           sT[j,i] = kT_j^T qT_i          (PE, contraction over features)
           eT = exp(sT * scale)           (ScalarE, PSUM->SBUF, bf16)
           sums[i] = eT^T @ ones          (PE, per-partition result)
           o[i,:] = (eT_ic^T @ v) / sums  (PE + VectorE normalize)
"""

import numpy as np
from contextlib import ExitStack

import concourse.bass as bass
import concourse.tile as tile
from concourse import bacc, mybir
from concourse.bass_utils import run_bass_kernel_spmd
from concourse.masks import make_identity

P = 128
FP32 = mybir.dt.float32
BF16 = mybir.dt.bfloat16
AF = mybir.ActivationFunctionType

B, S_FULL, D_FULL, F_FULL = 8, 2048, 1024, 1024
N_CORES = 8


def build_attention(nc, S, D, F, repeat=1):
    scale = 1.0 / float(np.sqrt(F))
    ND, NF, NS = D // P, F // P, S // P
    SS = 512                 # s / i super-block width
    NSS = S // SS
    NI = S // SS
    NJ = NS                  # j blocks of 128
    VCW = min(F, 512)        # vd chunk width
    NV = F // VCW
    NIC = SS // P            # i sub-chunks per i-block

    x = nc.dram_tensor("x", [S, D], FP32, kind="ExternalInput").ap()
    wq = nc.dram_tensor("wq", [D, F], FP32, kind="ExternalInput").ap()
    bq = nc.dram_tensor("bq", [F], FP32, kind="ExternalInput").ap()
    wk = nc.dram_tensor("wk", [D, F], FP32, kind="ExternalInput").ap()
    bk = nc.dram_tensor("bk", [F], FP32, kind="ExternalInput").ap()
    wv = nc.dram_tensor("wv", [D, F], FP32, kind="ExternalInput").ap()
    bv = nc.dram_tensor("bv", [F], FP32, kind="ExternalInput").ap()
    out = nc.dram_tensor("out", [S, F], FP32, kind="ExternalOutput").ap()

    with tile.TileContext(nc) as tc, ExitStack() as ctx:
        consts = ctx.enter_context(tc.tile_pool(name="consts", bufs=1))
        ident_bf = consts.tile([P, P], BF16)
        make_identity(nc, ident_bf)
        ones = consts.tile([P, 1], BF16)
        nc.vector.memset(ones, 1.0)
        bq_sb = consts.tile([P, NF], FP32)
        nc.sync.dma_start(bq_sb, bq.rearrange("(fo fi) -> fi fo", fi=P))
        bk_sb = consts.tile([P, NF], FP32)
        nc.sync.dma_start(bk_sb, bk.rearrange("(fo fi) -> fi fo", fi=P))
        bv_sb = consts.tile([P, F], FP32)
        nc.sync.dma_start(
            bv_sb,
            bass.AP(tensor=bv.tensor, offset=bv.offset, ap=[[0, P]] + list(bv.ap)),
        )

        kT = ctx.enter_context(tc.tile_pool(name="kTp", bufs=1)).tile([P, NF, S], BF16)
        qT = ctx.enter_context(tc.tile_pool(name="qTp", bufs=1)).tile([P, NF, S], BF16)
        vv = ctx.enter_context(tc.tile_pool(name="vp", bufs=1)).tile([P, NS, F], BF16)

        # ---------------- Phase 1: xT + projections ----------------
        def _phase1():
          with ExitStack() as ph1:
            wpool = ph1.enter_context(tc.tile_pool(name="wpool", bufs=1))
            wstage = ph1.enter_context(tc.tile_pool(name="wstage", bufs=2))
            w_sb = {}
            cast_engs = (nc.vector, nc.gpsimd)
            for wi, (name, wap) in enumerate((("wk", wk), ("wq", wq), ("wv", wv))):
                wt = wpool.tile([P, ND, F], BF16, tag=f"w_{name}")
                w_sb[name] = wt
                for do in range(ND):
                    st = wstage.tile([P, F], FP32, tag="wst")
                    nc.sync.dma_start(st, wap[do * P:(do + 1) * P, :])
                    cast_engs[(wi * ND + do) % 2].tensor_copy(out=wt[:, do, :], in_=st)

            xstage = ph1.enter_context(tc.tile_pool(name="xstage", bufs=4))
            xbstage = ph1.enter_context(tc.tile_pool(name="xbstage", bufs=4))
            xTpool = ph1.enter_context(tc.tile_pool(name="xTpool", bufs=2))
            ps_tr = ph1.enter_context(tc.tile_pool(name="ps_tr", bufs=3, space="PSUM"))
            ps_mm = ph1.enter_context(tc.tile_pool(name="ps_mm", bufs=4, space="PSUM"))

            nsb = SS // P
            for ss in range(NSS):
                xT = xTpool.tile([P, ND, SS], BF16, tag="xT")
                for sb in range(nsb):
                    si = ss * nsb + sb
                    xs = xstage.tile([P, D], FP32, tag="xs")
                    nc.sync.dma_start(xs, x[si * P:(si + 1) * P, :])
                    # cast to bf16 on GpSimd (idle engine) so the PE transposes
                    # run at 1 cyc/row instead of fp32's 2
                    xb = xbstage.tile([P, D], BF16, tag="xb")
                    nc.gpsimd.tensor_copy(out=xb, in_=xs)
                    for h0 in range(0, ND, 4):
                        g = min(4, ND - h0)
                        pt = ps_tr.tile([P, 4, P], BF16, tag="pt")
                        for c in range(g):
                            nc.tensor.transpose(
                                pt[:, c, :], xb[:, (h0 + c) * P:(h0 + c + 1) * P],
                                ident_bf,
                            )
                        dst = xT[:, h0:h0 + g, sb * P:(sb + 1) * P]
                        if si % 2 == 0:
                            nc.scalar.copy(out=dst, in_=pt[:, :g, :])
                        else:
                            nc.vector.tensor_copy(out=dst, in_=pt[:, :g, :])

                # kT / qT projections (feature-major)
                for name, dstT, bias_sb in (("wk", kT, bk_sb), ("wq", qT, bq_sb)):
                    for fo in range(NF):
                        pmm = ps_mm.tile([P, SS], FP32, tag="mm")
                        for do in range(ND):
                            nc.tensor.matmul(
                                pmm,
                                w_sb[name][:, do, fo * P:(fo + 1) * P],
                                xT[:, do, :],
                                start=(do == 0),
                                stop=(do == ND - 1),
                            )
                        nc.scalar.activation(
                            out=dstT[:, fo, ss * SS:(ss + 1) * SS],
                            in_=pmm,
                            func=AF.Identity,
                            bias=bias_sb[:, fo:fo + 1],
                            scale=1.0,
                        )
                # v projection (natural layout)
                for sb in range(nsb):
                    si = ss * nsb + sb
                    for vc in range(NV):
                        c0 = vc * VCW
                        pmm = ps_mm.tile([P, VCW], FP32, tag="mm")
                        for do in range(ND):
                            nc.tensor.matmul(
                                pmm,
                                xT[:, do, sb * P:(sb + 1) * P],
                                w_sb["wv"][:, do, c0:c0 + VCW],
                                start=(do == 0),
                                stop=(do == ND - 1),
                            )
                        nc.vector.tensor_add(
                            out=vv[:, si, c0:c0 + VCW],
                            in0=pmm,
                            in1=bv_sb[:, c0:c0 + VCW],
                        )

        # ---------------- Phase 2: attention ----------------
        def _phase2():
          with ExitStack() as ph2:
            eTpool = ph2.enter_context(tc.tile_pool(name="eTpool", bufs=2))
            rpool = ph2.enter_context(tc.tile_pool(name="rpool", bufs=2))
            ostage = ph2.enter_context(tc.tile_pool(name="ostage", bufs=3))
            ps_s = ph2.enter_context(tc.tile_pool(name="ps_s", bufs=2, space="PSUM"))
            ps_st = ph2.enter_context(tc.tile_pool(name="ps_st", bufs=2, space="PSUM"))
            ps_av = ph2.enter_context(tc.tile_pool(name="ps_av", bufs=4, space="PSUM"))

            for ib in range(NI):
                eT = eTpool.tile([P, NJ, SS], BF16, tag="eT")
                psumT = ps_st.tile([P, NIC], FP32, tag="sumT")
                for jb in range(NJ):
                    ps = ps_s.tile([P, SS], FP32, tag="s")
                    for fo in range(NF):
                        nc.tensor.matmul(
                            ps,
                            kT[:, fo, jb * P:(jb + 1) * P],
                            qT[:, fo, ib * SS:(ib + 1) * SS],
                            start=(fo == 0),
                            stop=(fo == NF - 1),
                        )
                    nc.scalar.activation(
                        out=eT[:, jb, :], in_=ps, func=AF.Exp, scale=scale
                    )
                    for ic in range(NIC):
                        # One PSUM accumulation group spans the whole [P, NIC]
                        # tile: start marks the full 2KB zero-region pending-
                        # zero, so each column's first write overwrites.
                        nc.tensor.matmul(
                            psumT[:, ic:ic + 1],
                            eT[:, jb, ic * P:(ic + 1) * P],
                            ones,
                            start=(jb == 0 and ic == 0),
                            stop=(jb == NJ - 1 and ic == NIC - 1),
                        )
                recip = rpool.tile([P, NIC], FP32, tag="recip")
                nc.vector.reciprocal(recip, psumT)
                for ic in range(NIC):
                    for vc in range(NV):
                        c0 = vc * VCW
                        po = ps_av.tile([P, VCW], FP32, tag="av")
                        for jb in range(NJ):
                            nc.tensor.matmul(
                                po,
                                eT[:, jb, ic * P:(ic + 1) * P],
                                vv[:, jb, c0:c0 + VCW],
                                start=(jb == 0),
                                stop=(jb == NJ - 1),
                            )
                        ot = ostage.tile([P, VCW], FP32, tag="ot")
                        nc.vector.tensor_scalar_mul(ot, po, recip[:, ic:ic + 1])
                        nc.sync.dma_start(
                            out[ib * SS + ic * P: ib * SS + (ic + 1) * P, c0:c0 + VCW],
                            ot,
                        )

        # `repeat` re-emits the whole computation; >1 used only for wall-clock
        # timing of the per-iteration device time.
        for _rep in range(repeat):
            _phase1()
            _phase2()
    return nc


_CACHE = {}


def _get_module():
    if "nc" not in _CACHE:
        nc = bacc.Bacc(
            "TRN2", target_bir_lowering=False, debug=False, num_devices=N_CORES
        )
        build_attention(nc, S_FULL, D_FULL, F_FULL)
        nc.compile()
        _CACHE["nc"] = nc
    return _CACHE["nc"]


def _in_maps(query, Wq, bq, Wk, bk, Wv, bv):
    def f32(a):
        return np.ascontiguousarray(np.asarray(a, dtype=np.float32))

    query, Wq, bq, Wk, bk, Wv, bv = map(f32, (query, Wq, bq, Wk, bk, Wv, bv))
    return [
        {
            "x": np.ascontiguousarray(query[b]),
            "wq": Wq,
            "bq": bq,
            "wk": Wk,
            "bk": bk,
            "wv": Wv,
            "bv": bv,
        }
        for b in range(B)
    ]


def kernel(query, Wq, bq, Wk, bk, Wv, bv):
    nc = _get_module()
    in_maps = _in_maps(query, Wq, bq, Wk, bk, Wv, bv)
    res = run_bass_kernel_spmd(nc, in_maps, core_ids=list(range(N_CORES)))
    return np.stack([r["out"] for r in res.results], axis=0)
